# revision 1
# baseline (speedup 1.0000x reference)
"""Trainium2 Bass kernel for GatedGraphXBias (gnn_message_passing), v2.

Reference math per iteration (T=2048 notes, E=12 edge types, H=64):
    act[e]  = edge[e].T @ h                      # [T, H]
    a       = sum_e (act[e] + ba[e]) @ W[e] + bw # [T, 3H] -> az|ar|ah
    a      += x @ Win                            # hoisted input projection
    z       = sigmoid(az + h @ Uz)
    r       = sigmoid(ar + h @ Ur)
    h~      = tanh(ah + (r*h) @ Uh)
    h       = (1-z)*h + z*h~

Sequence-parallel over T across 8 cores (TL=256 notes each).  Changes vs v1:
  * all matmul operands in fp16 (PSUM accumulation stays f32): halves edge
    SBUF/DMA footprint and the per-iteration AllGather payload.  (bf16 was
    tried first: its 8-bit mantissa cost 1.7e-2 rel err; fp16 gives 2.1e-3.)
  * mm1 is column-group tiled: edge-type pairs compute concurrently in PE
    array columns 0-63 / 64-127 (tile_position), with the pair's
    activations stacked on PSUM partitions [128, t].
  * mm2 contracts over the stacked pair (K=128, one matmul per gate per
    pair): 18 matmuls/iter instead of 36.
  * DMA pipelining: h loads ahead of the edge shard, edge chunks alternate
    the SP/ACT HWDGE rings behind a chunk-outer iteration-0 mm1, the h
    reload is split 4-way across both rings, iterations >=1 run slot-major
    so PSUM copies + mm2 overlap the mm1 stream.
  * ~5us of PE keepalive matmuls inside the collective window stop the
    activity monitor from re-throttling the clock across the gather gap.
  * gate tail: r-sigmoid issues before z and (1-z)*h is precomputed in the
    tanh's shadow, so only z*h~ and one add trail the tanh.
"""

import sys

sys.path.insert(0, "/opt/trn_rl_repo")

import numpy as np
import concourse.bass as bass
import concourse.mybir as mybir
import concourse.tile as tile
from concourse.bass_utils import run_bass_kernel_spmd
from concourse.masks import make_identity
from concourse.vector_clock import ScopedClock

E, T, H, IN = 12, 2048, 64, 128
M = 8  # cores
TL = T // M  # 256 local notes per core
NCH = T // 128  # 16 contraction chunks of 128 source notes
NSLOT = 3  # mm1 PSUM slots, each covering 2 edge-type pairs
F32 = mybir.dt.float32
F32R = mybir.dt.float32r
BF16 = mybir.dt.float16  # fp16: 11-bit mantissa (bf16's 8 was too coarse here)
SIG = mybir.ActivationFunctionType.Sigmoid
TANH = mybir.ActivationFunctionType.Tanh
# per-chunk edge column order: slot k holds [e=4k (A), e=4k+2 (A), e=4k+1 (B), e=4k+3 (B)]
E_ORDER = [0, 2, 1, 3, 4, 6, 5, 7, 8, 10, 9, 11]


class SplitDrainTileContext(tile.TileContext):
    """TileContext that limits every instruction to a single sync wait.

    This walrus build rejects >1 sync wait command on an instruction
    (setupSyncWait: "Too many sync wait commands"), so extra waits are
    peeled onto standalone same-engine NoOps emitted just before the
    instruction — semantically identical (the engine stream waits
    sequentially at the same program point)."""

    def _commit_instruction(self, inst, lazy_reg_writes: bool = True):
        si = getattr(inst, "sync_info", None)
        if si is not None and len(si.on_wait) > 1:
            waits = list(si.on_wait)
            inst.sync_info = mybir.SyncInfo(
                on_wait=[waits[-1]], on_update=list(si.on_update)
            )
            for w in waits[:-1]:
                nop = mybir.InstNoOp(
                    name=f"splitwait-{self.nc.next_id()}",
                    sync_info=mybir.SyncInfo(on_wait=[w], on_update=[]),
                    bass_nofuse=True,
                    engine=inst.engine,
                )
                super()._commit_instruction(nop, lazy_reg_writes=False)
        super()._commit_instruction(inst, lazy_reg_writes)

    def _drain_and_barrier(self, tick_clock, wait_clock):
        drain_inst = self.nc.sync.drain()
        wait_clock.add_sem_waits(
            drain_inst.ins, ScopedClock({None: tick_clock.global_clock})
        )
        si = drain_inst.ins.sync_info
        waits = list(si.on_wait) if si is not None else []
        upds = list(si.on_update) if si is not None else []
        if len(waits) > 1:
            drain_inst.ins.sync_info = mybir.SyncInfo(on_wait=waits[:1], on_update=upds)
            for w in waits[1:]:
                nop = self.nc.sync.nop(nofuse=True, hint="split_drain_waits")
                nop.ins.sync_info = mybir.SyncInfo(on_wait=[w], on_update=[])

        self.nc.all_engine_barrier()
        assert self.sems is not None
        popped = self.nc._tile_sem_poison_stack.pop()
        assert popped is self._sem_poison
        self.nc.clear_and_free_semaphores(list(self.sems.allocated().values()))
        self.nc.all_engine_barrier()


def build(iteration: int, reps: int = 1, ablate: frozenset = frozenset()) -> bass.Bass:
    nc = bass.Bass(
        "TRN2",
        target_bir_lowering=False,
        debug=False,
        num_devices=M,
        dynamic_dma_scratch_size=2048,
    )

    # Per-core inputs (host pre-arranged):
    #   edge_in : [T, E*TL] bf16, row = source note, col = chunk-local
    #             [slotk: eA0 | eA1 | eB0 | eB1] blocks of TL (see E_ORDER)
    #   h0sb_in : [128, NCH*H] bf16  initial h in chunk-major SBUF layout
    #   hid_in  : [T, H] bf16        initial h, note-major (ablate path)
    #   hT0_in  : [H, TL] f32        local initial hidden, transposed
    #   hT0b_in : [H, TL] bf16
    #   xT_in   : [IN, TL] bf16      local input features, transposed
    #   w_in    : [128, 6*3*H] bf16  pair-stacked W: col p*192+g*64+j is
    #             gate g of pair p; rows 0-63 = W[2p], 64-127 = W[2p+1]
    #   win_in  : [IN, 3H] bf16
    #   uzr_in  : [H, 2H] bf16
    #   uh_in   : [H, H] bf16
    #   bz/br/bh: [H, 1] f32         folded biases (bw + sum_e ba[e] @ W[e])
    edge_in = nc.declare_dram_parameter("edge_in", [T, E * TL], BF16, isOutput=False)
    h0sb_in = nc.declare_dram_parameter("h0sb_in", [128, NCH * H], BF16, isOutput=False)
    hid_in = nc.declare_dram_parameter("hid_in", [T, H], BF16, isOutput=False)
    hT0_in = nc.declare_dram_parameter("hT0_in", [H, TL], F32, isOutput=False)
    hT0b_in = nc.declare_dram_parameter("hT0b_in", [H, TL], BF16, isOutput=False)
    xT_in = nc.declare_dram_parameter("xT_in", [IN, TL], BF16, isOutput=False)
    w_in = nc.declare_dram_parameter("w_in", [128, 6 * 3 * H], BF16, isOutput=False)
    win_in = nc.declare_dram_parameter("win_in", [IN, 3 * H], BF16, isOutput=False)
    uzr_in = nc.declare_dram_parameter("uzr_in", [H, 2 * H], BF16, isOutput=False)
    uh_in = nc.declare_dram_parameter("uh_in", [H, H], BF16, isOutput=False)
    bz_in = nc.declare_dram_parameter("bz_in", [H, 1], F32, isOutput=False)
    br_in = nc.declare_dram_parameter("br_in", [H, 1], F32, isOutput=False)
    bh_in = nc.declare_dram_parameter("bh_in", [H, 1], F32, isOutput=False)
    h_out = nc.declare_dram_parameter("h_out", [TL, H], F32, isOutput=True)

    with SplitDrainTileContext(nc) as tc:
        with (
            tc.tile_pool(name="edge", bufs=1) as edge_pool,
            tc.tile_pool(name="const", bufs=1) as cpool,
            tc.tile_pool(name="work", bufs=1) as wpool,
            tc.tile_pool(name="psum", bufs=1, space="PSUM") as ppool,
            tc.tile_pool(name="dram", bufs=2, space="DRAM") as dpool,
        ):
            # ---- constants / weights (loaded once) ----
            w_sb = cpool.tile([128, 6 * 3 * H], BF16)
            nc.sync.dma_start(out=w_sb[:], in_=w_in[:])
            uzr_sb = cpool.tile([H, 2 * H], BF16)
            nc.sync.dma_start(out=uzr_sb[:], in_=uzr_in[:])
            uh_sb = cpool.tile([H, H], BF16)
            nc.sync.dma_start(out=uh_sb[:], in_=uh_in[:])
            win_sb = cpool.tile([IN, 3 * H], BF16)
            nc.sync.dma_start(out=win_sb[:], in_=win_in[:])
            xT_sb = cpool.tile([IN, TL], BF16)
            nc.sync.dma_start(out=xT_sb[:], in_=xT_in[:])
            bz_sb = cpool.tile([H, 1], F32)
            nc.sync.dma_start(out=bz_sb[:], in_=bz_in[:])
            br_sb = cpool.tile([H, 1], F32)
            nc.sync.dma_start(out=br_sb[:], in_=br_in[:])
            bh_sb = cpool.tile([H, 1], F32)
            nc.sync.dma_start(out=bh_sb[:], in_=bh_in[:])
            id64 = cpool.tile([H, H], F32)
            make_identity(nc, id64[:])
            id64b = cpool.tile([H, H], BF16)
            make_identity(nc, id64b[:])

            for rep in range(reps):
                # ---- h state first (so iteration 0 isn't queued behind the
                # edge load on the SP HWDGE ring) ----
                h_sb = wpool.tile([128, NCH * H], BF16, name="h", tag="h", bufs=1)
                nc.sync.dma_start(out=h_sb[:], in_=h0sb_in[:])
                hT_sb = wpool.tile([H, TL], F32, name="hT", tag="hT", bufs=2)
                nc.sync.dma_start(out=hT_sb[:], in_=hT0_in[:])
                hTb_sb = wpool.tile([H, TL], BF16, name="hTb", tag="hTb", bufs=2)
                nc.sync.dma_start(out=hTb_sb[:], in_=hT0b_in[:])

                # ---- resident edge shard: 16 chunk DMAs, alternating the
                # SP / ACT HWDGE rings so chunk c lands ~c*1.2us in and the
                # chunk-outer mm1 of iteration 0 streams right behind ----
                edge_sb = []
                for c in range(NCH):
                    et = edge_pool.tile(
                        [128, E * TL], BF16, name=f"edge_c{c}", tag=f"edge_c{c}"
                    )
                    edge_sb.append(et)
                for c in range(NCH):
                    eng = nc.sync if c % 2 == 0 else nc.scalar
                    eng.dma_start(
                        out=edge_sb[c][:],
                        in_=edge_in[c * 128 : (c + 1) * 128, :],
                    )

                for it in range(iteration):
                    last = it == iteration - 1

                    # Gate pre-activation groups [H, TL]: folded input
                    # projection starts each, U-gate matmuls fold in, then
                    # the 18 pair-stacked mm2 matmuls accumulate.
                    az_ps = ppool.tile([H, TL], F32, tag="az", bufs=1)
                    ar_ps = ppool.tile([H, TL], F32, tag="ar", bufs=1)
                    ah_ps = ppool.tile([H, TL], F32, tag="ah", bufs=1)
                    for g, ps in enumerate((az_ps, ar_ps, ah_ps)):
                        nc.tensor.matmul(
                            ps[:],
                            lhsT=win_sb[:, g * H : (g + 1) * H],
                            rhs=xT_sb[:],
                            start=True,
                            stop=False,
                            skip_group_check=True,
                        )
                    for g, ps in enumerate((az_ps, ar_ps)):
                        nc.tensor.matmul(
                            ps[:],
                            lhsT=uzr_sb[:, g * H : (g + 1) * H],
                            rhs=hTb_sb[:],
                            start=False,
                            stop="mm2" in ablate or "mm1" in ablate,
                            skip_group_check=True,
                        )

                    # -- mm1: chunk-outer, col-group tiled over e-pairs --
                    act_ps = []
                    for k in range(NSLOT):
                        act_k = ppool.tile(
                            [128, 2 * TL], F32, name=f"act{k}", tag=f"act{k}", bufs=1
                        )
                        act_ps.append(act_k)
                    if "mm1" not in ablate:
                        # iteration 0: chunk-outer (pipelines behind the edge
                        # load); later iterations: slot-outer, so slot 0's
                        # PSUM copy + mm2 overlap the remaining mm1 stream
                        if it == 0:
                            order = [(c, k) for c in range(NCH) for k in range(NSLOT)]
                        else:
                            order = [(c, k) for k in range(NSLOT) for c in range(NCH)]
                        for c, k in order:
                            lhs = h_sb[:, c * H : (c + 1) * H]
                            base = k * 4 * TL
                            nc.tensor.matmul(
                                act_ps[k][0:H, :],
                                lhsT=lhs,
                                rhs=edge_sb[c][:, base : base + 2 * TL],
                                start=(c == 0),
                                stop=(c == NCH - 1),
                                tile_position=(0, 0),
                                skip_group_check=True,
                            )
                            nc.tensor.matmul(
                                act_ps[k][H:128, :],
                                lhsT=lhs,
                                rhs=edge_sb[c][:, base + 2 * TL : base + 4 * TL],
                                start=(c == 0),
                                stop=(c == NCH - 1),
                                tile_position=(0, H),
                                skip_group_check=True,
                            )

                        # -- act PSUM -> SBUF (bf16) + mm2 --
                        for k in range(NSLOT):
                            act_sb = wpool.tile(
                                [128, 2 * TL], BF16, tag=f"actsb{k}", bufs=2
                            )
                            if k == 1:
                                nc.scalar.activation(
                                    act_sb[:],
                                    act_ps[k][:],
                                    mybir.ActivationFunctionType.Copy,
                                )
                            else:
                                nc.vector.tensor_copy(act_sb[:], act_ps[k][:])
                            if "mm2" in ablate:
                                continue
                            for j in range(2):
                                p = 2 * k + j
                                rhs = act_sb[:, j * TL : (j + 1) * TL]
                                for g, ps in enumerate((az_ps, ar_ps, ah_ps)):
                                    nc.tensor.matmul(
                                        ps[:],
                                        lhsT=w_sb[
                                            :,
                                            p * 3 * H + g * H : p * 3 * H + (g + 1) * H,
                                        ],
                                        rhs=rhs,
                                        start=False,
                                        stop=(p == 5 and g != 2),
                                        skip_group_check=True,
                                    )
                    # -- gates --  (r first: it heads the rh -> Uh -> tanh
                    # critical chain; z's products are precomputed in its
                    # shadow so only 2 DVE ops remain after the tanh)
                    r_sb = wpool.tile([H, TL], F32, tag="r")
                    nc.scalar.activation(r_sb[:], ar_ps[:], SIG, bias=br_sb[:])
                    z_sb = wpool.tile([H, TL], F32, tag="z")
                    nc.scalar.activation(z_sb[:], az_ps[:], SIG, bias=bz_sb[:])
                    rh_sb = wpool.tile([H, TL], BF16, tag="rh")
                    nc.vector.tensor_mul(rh_sb[:], r_sb[:], hT_sb[:])
                    nc.tensor.matmul(
                        ah_ps[:],
                        lhsT=uh_sb[:],
                        rhs=rh_sb[:],
                        start=False,
                        stop=True,
                        skip_group_check=True,
                    )
                    omz_sb = wpool.tile([H, TL], F32, tag="omz")
                    nc.vector.tensor_scalar(
                        omz_sb[:], z_sb[:], -1.0, 1.0,
                        mybir.AluOpType.mult, mybir.AluOpType.add,
                    )  # 1-z
                    omzh_sb = wpool.tile([H, TL], F32, tag="omzh")
                    nc.vector.tensor_mul(omzh_sb[:], omz_sb[:], hT_sb[:])  # (1-z)*h
                    ht_sb = wpool.tile([H, TL], F32, tag="ht")
                    nc.scalar.activation(ht_sb[:], ah_ps[:], TANH, bias=bh_sb[:])

                    zd_sb = wpool.tile([H, TL], F32, tag="zd")
                    nc.vector.tensor_mul(zd_sb[:], z_sb[:], ht_sb[:])  # z*h~
                    hnewT_sb = wpool.tile([H, TL], F32, tag="hT", bufs=2)
                    nc.vector.tensor_add(hnewT_sb[:], zd_sb[:], omzh_sb[:])

                    if last:
                        # transpose f32 -> [TL, H] and store the output
                        hnew_sb = wpool.tile([128, 2 * H], F32, tag="hnew")
                        for half in range(2):
                            tr_ps = ppool.tile([128, H], F32, tag="tr")
                            nc.tensor.transpose(
                                tr_ps[:],
                                hnewT_sb[:, half * 128 : (half + 1) * 128],
                                id64[:],
                            )
                            nc.vector.tensor_copy(
                                hnew_sb[:, half * H : (half + 1) * H], tr_ps[:]
                            )
                        nc.sync.dma_start(
                            out=h_out[:].rearrange("(c p) j -> p c j", p=128),
                            in_=hnew_sb[:].rearrange("p (c j) -> p c j", c=2),
                        )
                    else:
                        # fp16 hT copy for the next iteration's U-gate matmul
                        # (off the critical path: the gather payload below
                        # transposes the f32 hnewT directly)
                        hnTb_sb = wpool.tile([H, TL], BF16, tag="hTb", bufs=2)
                        nc.vector.tensor_copy(hnTb_sb[:], hnewT_sb[:])
                        hnew_sb = wpool.tile([128, 2 * H], BF16, tag="hnewb")
                        for half in range(2):
                            tr_ps = ppool.tile([128, H], F32, tag="tr")
                            nc.tensor.transpose(
                                tr_ps[:],
                                hnewT_sb[:, half * 128 : (half + 1) * 128],
                                id64[:],
                            )
                            nc.vector.tensor_copy(
                                hnew_sb[:, half * H : (half + 1) * H], tr_ps[:]
                            )

                        if "coll" not in ablate:
                            ag_in = dpool.tile([TL, H], BF16, tag="ag_in")
                            for half in range(2):
                                # one store per HWDGE ring so they run in
                                # parallel (both on one ring would serialize
                                # ~0.6us each on the pre-collective path)
                                eng = nc.sync if half == 0 else nc.scalar
                                eng.dma_start(
                                    out=ag_in[half * 128 : (half + 1) * 128, :],
                                    in_=hnew_sb[:, half * H : (half + 1) * H],
                                )
                            ag_out = dpool.tile(
                                [T, H], BF16, tag="ag_out", addr_space="Shared"
                            )
                            nc.gpsimd.collective_compute(
                                "AllGather",
                                mybir.AluOpType.bypass,
                                replica_groups=[list(range(M))],
                                ins=[ag_in[:]],
                                outs=[ag_out[:]],
                            )
                            gather_src = ag_out
                            warm_ps = ppool.tile(
                                [H, 2 * TL], F32, name="warm", tag="warm", bufs=1
                            )
                            for _ in range(24):
                                nc.tensor.matmul(
                                    warm_ps[:],
                                    lhsT=hnew_sb[:, 0:H],
                                    rhs=edge_sb[0][:, 0 : 2 * TL],
                                    start=True,
                                    stop=True,
                                    skip_group_check=True,
                                )
                        else:
                            gather_src = hid_in
                        if "hreload" not in ablate:
                            h_sb = wpool.tile([128, NCH * H], BF16, tag="h", bufs=1)
                            qch = NCH // 4
                            for qq in range(4):
                                eng = nc.sync if qq % 2 == 0 else nc.scalar
                                eng.dma_start(
                                    out=h_sb[
                                        :, qq * qch * H : (qq + 1) * qch * H
                                    ].rearrange("p (c j) -> p c j", c=qch),
                                    in_=gather_src[
                                        qq * qch * 128 : (qq + 1) * qch * 128, :
                                    ].rearrange("(c p) j -> p c j", p=128),
                                )
                        hT_sb = hnewT_sb
                        hTb_sb = hnTb_sb

    return nc


def _host_prep(input, hidden, edge_matrix, ba, wz_wr_wh, uz_ur, uh, input_wzrh, bw):
    """Pre-arrange full inputs into the per-core DMA layouts."""
    bf = mybir.dt.np(BF16)
    x = np.asarray(input, np.float32)[0]  # [T, IN]
    h0 = np.ascontiguousarray(np.asarray(hidden, np.float32)[0])  # [T, H]
    edge = np.asarray(edge_matrix, np.float32)  # [E, T, T]
    ba = np.asarray(ba, np.float32)
    W = np.asarray(wz_wr_wh, np.float32)  # [E, H, 3H]
    uzr = np.ascontiguousarray(np.asarray(uz_ur, np.float32))
    uh_ = np.ascontiguousarray(np.asarray(uh, np.float32))
    win = np.ascontiguousarray(np.asarray(input_wzrh, np.float32))
    bw = np.asarray(bw, np.float32)

    # folded bias: bw + sum_e ba[e] @ W[e]
    btot = bw + np.einsum("eh,ehk->k", ba, W)  # [3H]
    bz = np.ascontiguousarray(btot[:H].reshape(H, 1))
    br = np.ascontiguousarray(btot[H : 2 * H].reshape(H, 1))
    bh = np.ascontiguousarray(btot[2 * H :].reshape(H, 1))

    # edge shards: per core m, [T, E*TL] with columns in E_ORDER blocks
    # esh[m][s, i*TL + tl] = edge[E_ORDER[i], s, m*TL + tl]
    eperm = edge[E_ORDER]  # [E, T, T]
    esh = np.ascontiguousarray(
        eperm.reshape(E, T, M, TL).transpose(2, 1, 0, 3)
    ).reshape(M, T, E * TL).astype(bf)

    # pair-stacked W: w_flat[row, p*192 + g*64 + j]
    #   rows 0-63 = W[2p][:, g*64+j], rows 64-127 = W[2p+1][:, g*64+j]
    w_pair = np.empty((128, 6, 3 * H), np.float32)
    for p in range(6):
        w_pair[:H, p] = W[2 * p]
        w_pair[H:, p] = W[2 * p + 1]
    w_flat = np.ascontiguousarray(w_pair.reshape(128, 6 * 3 * H)).astype(bf)

    h0sb = np.ascontiguousarray(
        h0.reshape(NCH, 128, H).transpose(1, 0, 2).reshape(128, NCH * H)
    ).astype(bf)

    in_maps = []
    for m in range(M):
        xT = np.ascontiguousarray(x[m * TL : (m + 1) * TL, :].T)
        hT0 = np.ascontiguousarray(h0[m * TL : (m + 1) * TL, :].T)
        in_maps.append(
            {
                "edge_in": esh[m],
                "h0sb_in": h0sb,
                "hid_in": h0.astype(bf),
                "hT0_in": hT0,
                "hT0b_in": hT0.astype(bf),
                "xT_in": xT.astype(bf),
                "w_in": w_flat,
                "win_in": win.astype(bf),
                "uzr_in": uzr.astype(bf),
                "uh_in": uh_.astype(bf),
                "bz_in": bz,
                "br_in": br,
                "bh_in": bh,
            }
        )
    return in_maps


_NC_CACHE: dict = {}


def _get_nc(iteration: int, reps: int = 1, ablate: frozenset = frozenset()) -> bass.Bass:
    key = (iteration, reps, ablate)
    if key not in _NC_CACHE:
        _NC_CACHE[key] = build(iteration, reps=reps, ablate=ablate)
    return _NC_CACHE[key]


def kernel(
    input,
    hidden,
    edge_matrix,
    ba,
    wz_wr_wh,
    uz_ur,
    uh,
    input_wzrh,
    bw,
    iteration,
):
    iteration = int(iteration)
    if iteration <= 0:
        return np.asarray(hidden, np.float32).copy()

    nc = _get_nc(iteration)
    in_maps = _host_prep(
        input, hidden, edge_matrix, ba, wz_wr_wh, uz_ur, uh, input_wzrh, bw
    )
    res = run_bass_kernel_spmd(nc, in_maps, list(range(M)))
    out = np.concatenate([res.results[m]["h_out"] for m in range(M)], axis=0)
    return out[None]

